# revision 88
# baseline (speedup 1.0000x reference)
"""Trainium2 Bass kernel for nn_BottleneckTransformer.

Data-parallel over batch: B=16 samples -> 8 cores x 2 samples.
Per-core pipeline (per sample):
  A: conv1x1+BN1+relu (r), q/k projections, v^T projection (transposed layout)
  B: attention per head: scores^T = k^T q (K=64), exp on ACT, PV matmul with
     ones-augmented v^T (sumexp via extra column), normalize via
     reciprocal_approx_fast + gpsimd partition_broadcast
  C: conv1x1+BN2 (z), CBAM channel attention (PE matvecs + sigmoid),
     CBAM spatial attention (banded-matrix matmul formulation of the 7x7
     conv), residual + relu.

Matmul dtypes: float32r (tf32) for projections/scores (host-rounded inputs),
bf16 for exp_t/vT/PV (error damped by gamma~0.05), fp32 for tiny CBAM mms.
"""
import numpy as np

import concourse.bacc as bacc
import concourse.bass as bass
import concourse.tile as tile
from concourse import mybir, bass_isa
from concourse.bass_utils import run_bass_kernel_spmd

F32 = mybir.dt.float32
F32R = mybir.dt.float32r
BF16 = mybir.dt.bfloat16
FP8 = mybir.dt.float8e4
U8 = mybir.dt.uint8

# Schraudolph-style exp on DVE: bits = round(score*log2e*0.125*8 + EXP_BIAS)
# bitcast uint8 -> fp8e4m3 approximates exp(score*0.125).
EXP_SCALE = 1.4426950408889634
EXP_BIAS = 55.654

B, C, H, W = 16, 256, 32, 32
N = H * W          # 1024
NCORES = 8
SPC = B // NCORES  # samples per core = 2
NH, D = 4, 64      # heads, head dim
R = C // 8         # 32, channel attention bottleneck
EPS = 1e-5


def tf32_round(x):
    """Round fp32 -> tf32 (10-bit mantissa), round-to-nearest-even."""
    xi = np.ascontiguousarray(x, dtype=np.float32).view(np.uint32)
    lsb = (xi >> np.uint32(13)) & np.uint32(1)
    xi = xi + np.uint32(0x0FFF) + lsb
    xi &= np.uint32(0xFFFFE000)
    return xi.view(np.float32)


def build_module():
    nc = bacc.Bacc("TRN2", target_bir_lowering=False, debug=False)

    def din(name, shape, dt=F32):
        return nc.dram_tensor(name, shape, dt, kind="ExternalInput").ap()

    def dout(name, shape, dt=F32):
        return nc.dram_tensor(name, shape, dt, kind="ExternalOutput").ap()

    xr = din("xr", (SPC, 2, 128, N), F32R)        # per-sample x, c-blocks
    # wall[p, j, kb, c]: j = w1f, wq, wk, wv(gamma), w2f; [c,128 -> o cols]
    wall = din("wall", (128, 5, 2, C), F32R)
    # smalls[p, j]: sh1(2), bq(2), bk(2), sh2(2), ones(1), zeros(1)
    smalls = din("smalls", (128, 10), F32R)
    bv_r = din("bv_r", (1, C), F32)               # gamma folded, row
    caw1T = din("caw1T", (2, 128, 2 * R), F32)    # cols 0:32 avg(/1024), 32:64 max
    cwb = din("cwb", (R, C + 14 * 32), F32)       # caw2T ++ wband rows
    ident = din("ident", (128, 128), BF16)        # PE transpose identity

    out = dout("out", (SPC, 2, 128, N), F32)

    with tile.TileContext(nc) as tc:
        with (
            tc.tile_pool(name="wpool", bufs=1) as wp,
            tc.tile_pool(name="xpool", bufs=1) as xp,
            tc.tile_pool(name="rpool", bufs=1) as rp,
            tc.tile_pool(name="qkpool", bufs=1) as qkp,
            tc.tile_pool(name="vpool", bufs=1) as vp,
            tc.tile_pool(name="epool", bufs=14) as ep,
            tc.tile_pool(name="ypool", bufs=1) as yp,
            tc.tile_pool(name="zpool", bufs=2) as zp,
            tc.tile_pool(name="spool", bufs=2) as sp,
            tc.tile_pool(name="opool", bufs=2) as op_,
            tc.tile_pool(name="ps_sc", bufs=2, space="PSUM") as ps_sc,
            tc.tile_pool(name="ps_at", bufs=1, space="PSUM") as ps_at,
            tc.tile_pool(name="ps_a", bufs=2, space="PSUM") as ps_a,
        ):
            # ---- loads: spread across SP/ACT/DVE/Pool DMA queues so the
            # startup is not serialized on one HWDGE ring. conv1's needs
            # (w1, smalls, first x chunks) land first.
            idn = wp.tile([128, 128], BF16, tag="idn", name="idn")
            nc.scalar.dma_start(out=idn, in_=ident)
            wallt = wp.tile([128, 5, 2, C], F32R, tag="wallt", name="wallt")
            w1t, wqt, wkt, wvt, w2t = (wallt[:, j] for j in range(5))
            nc.scalar.dma_start(out=wallt[:, 0:1], in_=wall[:, 0:1])
            smt = wp.tile([128, 10], F32R, tag="smt", name="smt")
            nc.scalar.dma_start(out=smt, in_=smalls)
            sh1, bqc, bkc, sh2 = (smt.bitcast(F32)[:, 2 * j:2 * j + 2]
                                  for j in range(4))
            ones_fr = smt[:, 8:9]
            zero_fr = smt[:, 9:10]
            xt_all = [[xp.tile([128, N], F32R, tag=f"x{si}{cb}", name=f"x{si}{cb}")
                       for cb in range(2)] for si in range(SPC)]
            for nch in range(2):
                for cb in range(2):
                    nc.sync.dma_start(
                        out=xt_all[0][cb][:, nch * 512:(nch + 1) * 512],
                        in_=xr[0, cb][:, nch * 512:(nch + 1) * 512])
            nc.scalar.dma_start(out=wallt[:, 1:3], in_=wall[:, 1:3])
            nc.gpsimd.dma_start(out=wallt[:, 3:5], in_=wall[:, 3:5])
            for cb in range(2):
                nc.sync.dma_start(out=xt_all[1][cb], in_=xr[1, cb])
            cw1 = wp.tile([128, 2, 2 * R], F32, tag="cw1", name="cw1")
            nc.gpsimd.dma_start(out=cw1, in_=caw1T.rearrange("k p c -> p k c"))
            cwbt = wp.tile([R, C + 14 * 32], F32, tag="cwbt", name="cwbt")
            nc.gpsimd.dma_start(out=cwbt, in_=cwb)
            cw2 = cwbt[:, 0:C]
            wbd = cwbt[:, C:].rearrange("p (a b) -> p a b", a=14)
            bvb = wp.tile([128, C], F32, tag="bvb", name="bvb")
            bv_bc = bass.AP(tensor=bv_r.tensor, offset=bv_r.offset,
                            ap=[[0, 128]] + list(bv_r.ap)[1:])
            nc.gpsimd.dma_start(out=bvb, in_=bv_bc)

            # PE p-state warmup: harmless matmuls on the identity tile keep
            # the tensor engine continuously busy until conv1's inputs land,
            # so conv1 runs at full clock instead of ramping.
            wps = ps_a.tile([128, 512], F32, tag="pa", name="pa")
            NWARM = 32
            for i in range(NWARM):
                nc.tensor.matmul(wps[:, 0:128], idn, idn,
                                 start=(i == 0), stop=(i == NWARM - 1))

            xt = [None] * SPC      # [s][cb] f32r input tiles
            rt = [None] * SPC      # relu(conv1) tiles
            qt = [None] * SPC
            kt = [None] * SPC
            vt = [None] * SPC      # vT_aug bf16 [128, mb, 4*65]
            ytmp = [None] * SPC    # attn accum, then y = attn + r (f32r)

            def a_conv(s):
                xt[s] = xt_all[s]
                rt[s] = [rp.tile([128, N], F32R, tag=f"r{s}{ob}", name=f"r{s}{ob}")
                         for ob in range(2)]
                for ob in range(2):
                    for nch in range(2):
                        pa = ps_a.tile([128, 512], F32, tag="pa", name="pa")
                        for kb in range(2):
                            nc.tensor.matmul(
                                pa, w1t[:, kb, ob * 128:(ob + 1) * 128],
                                xt[s][kb][:, nch * 512:(nch + 1) * 512],
                                start=(kb == 0), stop=(kb == 1))
                        if s == 0:
                            # split startup evictions across ACT and DVE
                            nc.scalar.activation(
                                out=rt[s][ob][:, nch * 512:(nch + 1) * 512],
                                in_=pa, bias=sh1[:, ob:ob + 1],
                                func=mybir.ActivationFunctionType.Relu)
                        else:
                            nc.vector.tensor_scalar(
                                rt[s][ob][:, nch * 512:(nch + 1) * 512], pa,
                                sh1[:, ob:ob + 1], 0.0,
                                mybir.AluOpType.add, mybir.AluOpType.max)

            def a_proj(s, dst, wt, bc, on_act=False, flip=0):
                # dst: [128(h,dlo), 2(dhi), N] fp8 for DoubleRow scores.
                # wt cols are host-permuted so out partition = h*32+dlo.
                for sub in range(2):
                    for nch in range(2):
                        pa = ps_a.tile([128, 512], F32, tag="pa", name="pa")
                        for kb in range(2):
                            nc.tensor.matmul(
                                pa, wt[:, kb, sub * 128:(sub + 1) * 128],
                                rt[s][kb][:, nch * 512:(nch + 1) * 512],
                                start=(kb == 0), stop=(kb == 1))
                        if on_act:
                            nc.scalar.activation(
                                out=dst[:, sub, nch * 512:(nch + 1) * 512],
                                in_=pa, bias=bc[:, sub:sub + 1], scale=1.0,
                                func=mybir.ActivationFunctionType.Identity)
                        else:
                            nc.vector.tensor_scalar(
                                dst[:, sub, nch * 512:(nch + 1) * 512], pa,
                                bc[:, sub:sub + 1], 0.0,
                                mybir.AluOpType.add, mybir.AluOpType.add)

            def a_q(s):
                qt[s] = qkp.tile([128, 2, N], FP8, tag=f"q{s}", name=f"q{s}")
                a_proj(s, qt[s], wqt, bqc, on_act=(s == 0), flip=0)

            def a_k(s):
                kt[s] = qkp.tile([128, 2, N], FP8, tag=f"k{s}", name=f"k{s}")
                a_proj(s, kt[s], wkt, bkc, on_act=(s == 0), flip=1)

            def a_vt(s):
                # [128, mb, head, 80]: strides 16-aligned for DoubleRow lhsT
                vt[s] = vp.tile([128, 8, NH, 80], FP8, tag=f"v{s}", name=f"v{s}")
                for mb in range(8):
                    pa = ps_a.tile([128, 512], F32, tag="pa", name="pa")
                    for kb in range(2):
                        nc.tensor.matmul(
                            pa[:, 0:C],
                            rt[s][kb][:, mb * 128:(mb + 1) * 128],
                            wvt[:, kb, :], start=(kb == 0), stop=(kb == 1))
                    nc.vector.tensor_tensor(
                        out=vt[s][:, mb, :, 0:D],
                        in0=pa[:, 0:C].rearrange("p (h d) -> p h d", h=NH),
                        in1=bvb.rearrange("p (h d) -> p h d", h=NH),
                        op=mybir.AluOpType.add)
                nc.vector.memset(vt[s][:, :, :, D:D + 1], 1.0)

            def phase_a(s):
                # v^T is deferred into phase_b (pre hook): scores only need
                # conv1 + q + k, so exp work starts sooner.
                a_conv(s)
                a_q(s)
                a_k(s)

            def phase_b(s, extras=()):
                ytmp[s] = [yp.tile([128, N], F32R, tag=f"yt{s}{pb}", name=f"yt{s}{pb}")
                           for pb in range(2)]
                et_all = [[None] * 4 for _ in range(NH)]

                def emit_se(h, mb):
                    hs = slice(h * 32, (h + 1) * 32)
                    psc = ps_sc.tile([128, 1024], F32, tag="psc", name="psc")
                    for nch in range(2):
                        nc.tensor.matmul(
                            psc[:, nch * 512:(nch + 1) * 512],
                            kt[s][hs, :, mb * 128:(mb + 1) * 128],
                            qt[s][hs, :, nch * 512:(nch + 1) * 512],
                            start=True, stop=True,
                            perf_mode=mybir.MatmulPerfMode.DoubleRow,
                            tile_position=(h * 32, 0))
                    if mb % 2 == 0:
                        et_all[h][mb // 2] = ep.tile(
                            [128, 2, 1024], FP8, tag="et", name="et")
                    e = et_all[h][mb // 2]
                    if mb in (0, 5):
                        # bit-trick exp on DVE to offload the ACT engine
                        nc.vector.tensor_scalar(
                            e[:, mb % 2, :].bitcast(U8), psc,
                            EXP_SCALE, EXP_BIAS,
                            mybir.AluOpType.mult, mybir.AluOpType.add)
                    else:
                        nc.scalar.activation(
                            out=e[:, mb % 2, :], in_=psc,
                            func=mybir.ActivationFunctionType.Exp, scale=0.125)

                def emit_se_pair2(hp, mb):
                    # heads 2hp (rows 0-63) and 2hp+1 (rows 64-127):
                    # alternate MMs so adjacent instructions use different
                    # PE row groups and overlap on hardware
                    pb = hp
                    pscs = []
                    for j in range(2):
                        pscs.append(ps_sc.tile([128, 1024], F32, tag="psc",
                                               name="psc"))
                    for nch in range(2):
                        for j in range(2):
                            dsl = slice(j * 64, j * 64 + 64)
                            nc.tensor.matmul(
                                pscs[j][:, nch * 512:(nch + 1) * 512],
                                kt[s][pb][dsl, mb * 128:(mb + 1) * 128],
                                qt[s][pb][dsl, nch * 512:(nch + 1) * 512],
                                start=True, stop=True)
                    for j in range(2):
                        e = ep.tile([128, 1024], FP8, tag="et", name="et")
                        nc.scalar.activation(
                            out=e, in_=pscs[j],
                            func=mybir.ActivationFunctionType.Exp, scale=0.125)
                        et_all[2 * hp + j][mb] = e

                ytp = [None, None]

                def emit_pv(h):
                    # PV transposed: patT[n, d] per 128-n block; sumexp in
                    # col D; normalize per-partition; PE-transpose back.
                    pb, hh = h // 2, h % 2
                    et = et_all[h]
                    yT = sp.tile([128, 8, D], BF16, tag="yT", name="yT",
                                 bufs=2)
                    for half in range(2):
                        patT = ps_at.tile([128, 512], F32, tag="pat",
                                          name="pat")
                        for blk4 in range(4):
                            blk = half * 4 + blk4
                            for pr in range(4):
                                nc.tensor.matmul(
                                    patT[:, blk4 * 128:blk4 * 128 + D + 1],
                                    et[pr][:, :, blk * 128:(blk + 1) * 128],
                                    vt[s][:, 2 * pr:2 * pr + 2, h, 0:D + 1],
                                    start=(pr == 0), stop=(pr == 3),
                                    perf_mode=mybir.MatmulPerfMode.DoubleRow)
                        pv = patT.rearrange("p (b c) -> p b c", b=4)
                        rt4 = sp.tile([128, 4], F32, tag="rt4", name="rt4",
                                      bufs=4)
                        nc.vector.reciprocal_approx_fast(
                            out=rt4, in_=pv[:, :, D])
                        rb = bass.AP(tensor=rt4.tensor, offset=rt4.offset,
                                     ap=list(rt4.ap) + [[0, D]])
                        nc.vector.tensor_tensor(
                            out=yT[:, half * 4:(half + 1) * 4, :],
                            in0=pv[:, :, 0:D], in1=rb,
                            op=mybir.AluOpType.mult)
                    if hh == 0:
                        ytp[pb] = ps_at.tile([128, N], BF16, tag="ytp",
                                             name="ytp", bufs=1)
                    for blk in range(8):
                        nc.tensor.transpose(
                            out=ytp[pb][hh * 64:hh * 64 + 64,
                                        blk * 128:(blk + 1) * 128],
                            in_=yT[:, blk, :], identity=idn)

                PF = 8
                for h in range(NH):
                    for mb in (range(PF, 8) if h > 0 else range(8)):
                        emit_se(h, mb)
                    if h == 0 and vt[s] is None:
                        a_vt(s)  # overlap v^T with the first exp batch
                    if h + 1 < NH:
                        for mb in range(PF):
                            emit_se(h + 1, mb)
                    emit_pv(h)
                    if h % 2 == 1:
                        # y = attn + r (rounded to f32r for conv2)
                        pb = h // 2
                        nc.vector.tensor_tensor(
                            out=ytmp[s][pb],
                            in0=ytp[pb],
                            in1=rt[s][pb].bitcast(F32),
                            op=mybir.AluOpType.add)
                    if extras and h < len(extras):
                        extras[h]()

            def c_conv(s):
                # conv2 + bn2 -> z (fp32), with per-channel sums for CBAM avg.
                # Per-chunk sum accum + max reduce so the pools pipeline with
                # the matmuls instead of serializing after the full tile.
                zt = [zp.tile([128, N], F32R, tag=f"z{ob}", name=f"z{ob}")
                      for ob in range(2)]
                cols = [zp.tile([128, 4], F32, tag=f"cols{ob}", name=f"cols{ob}")
                        for ob in range(2)]
                for ob in range(2):
                    for nch in range(2):
                        pa = ps_a.tile([128, 512], F32, tag="pa", name="pa")
                        for kb in range(2):
                            nc.tensor.matmul(
                                pa, w2t[:, kb, ob * 128:(ob + 1) * 128],
                                ytmp[s][kb][:, nch * 512:(nch + 1) * 512],
                                start=(kb == 0), stop=(kb == 1))
                        if s == 1 and ob == 0:
                            nc.scalar.activation(
                                out=zt[ob][:, nch * 512:(nch + 1) * 512],
                                in_=pa, bias=sh2[:, ob:ob + 1],
                                func=mybir.ActivationFunctionType.Identity,
                                accum_out=cols[ob][:, nch:nch + 1])
                        else:
                            nc.vector.tensor_scalar(
                                zt[ob][:, nch * 512:(nch + 1) * 512], pa,
                                sh2[:, ob:ob + 1], 0.0,
                                mybir.AluOpType.add, mybir.AluOpType.add,
                                accum_out=cols[ob][:, nch:nch + 1])
                        nc.vector.tensor_reduce(
                            out=cols[ob][:, 2 + nch:3 + nch],
                            in_=zt[ob][:, nch * 512:(nch + 1) * 512],
                            op=mybir.AluOpType.max,
                            axis=mybir.AxisListType.X)
                    # combine chunk sums into col 0, chunk maxes into col 1
                    nc.vector.tensor_tensor(
                        out=cols[ob][:, 0:1], in0=cols[ob][:, 0:1],
                        in1=cols[ob][:, 1:2], op=mybir.AluOpType.add)
                    nc.vector.tensor_tensor(
                        out=cols[ob][:, 1:2], in0=cols[ob][:, 2:3],
                        in1=cols[ob][:, 3:4], op=mybir.AluOpType.max)
                return zt, cols

            def c_chan(s, zt, cols):
                # channel attention: h = relu(W1a@sum) + relu(W1m@max),
                # ca = sig(W2@h)
                ph = ps_a.tile([128, 512], F32, tag="pa", name="pa")
                for j in range(2):
                    for kb in range(2):
                        nc.tensor.matmul(ph[0:R, j:j + 1],
                                         cw1[:, kb, j * R:(j + 1) * R],
                                         cols[kb][:, j:j + 1],
                                         start=(kb == 0), stop=(kb == 1))
                hsb = sp.tile([R, 3], F32, tag="hsb", name="hsb")
                nc.vector.tensor_scalar(hsb[:, 0:2], ph[0:R, 0:2], 0.0, None,
                                        mybir.AluOpType.max)
                nc.vector.tensor_tensor(out=hsb[:, 2:3], in0=hsb[:, 0:1],
                                        in1=hsb[:, 1:2],
                                        op=mybir.AluOpType.add)
                ca = [sp.tile([128, 1], F32R, tag=f"ca{ob}", name=f"ca{ob}")
                      for ob in range(2)]
                pc = ps_a.tile([128, 512], F32, tag="pa", name="pa")
                for ob in range(2):
                    nc.tensor.matmul(pc[:, ob:ob + 1],
                                     cw2[:, ob * 128:(ob + 1) * 128],
                                     hsb[:, 2:3], start=True, stop=True)
                for ob in range(2):
                    nc.scalar.activation(
                        out=ca[ob], in_=pc[:, ob:ob + 1],
                        func=mybir.ActivationFunctionType.Sigmoid)
                # apply channel attention -> z_ca (f32r for the ones-matmul).
                # ob=0 on DVE, ob=1 on ACT (Copy with per-partition scale AP)
                # so the two run in parallel.
                zca = [zp.tile([128, N], F32R, tag=f"zca{ob}", name=f"zca{ob}")
                       for ob in range(2)]
                nc.vector.tensor_scalar_mul(zca[0], zt[0],
                                            ca[0].bitcast(F32))
                nc.scalar.activation(
                    out=zca[1], in_=zt[1],
                    func=mybir.ActivationFunctionType.Copy,
                    scale=ca[1].bitcast(F32))
                return zca, ca

            def c_spat(s, zt, zca, ca):
                # spatial sum (avg path): ca^T @ z, so it does not wait on
                # the zca tiles
                avg_row = sp.tile([1, N], F32, tag="avg_row", name="avg_row", bufs=1)
                for nch in range(2):
                    psr = ps_a.tile([128, 512], F32, tag="pa", name="pa")
                    for kb in range(2):
                        nc.tensor.matmul(
                            psr[0:1, :],
                            ca[kb], zt[kb][:, nch * 512:(nch + 1) * 512],
                            start=(kb == 0), stop=(kb == 1))
                    nc.scalar.copy(
                        avg_row[:, nch * 512:(nch + 1) * 512], psr[0:1, :])
                # spatial max via TT max + gpsimd cross-partition reduce
                m1 = zp.tile([128, N], F32, tag="m1", name="m1", bufs=1)
                nc.vector.tensor_tensor(out=m1, in0=zca[0].bitcast(F32),
                                        in1=zca[1].bitcast(F32),
                                        op=mybir.AluOpType.max)
                rep = zp.tile([128, N], F32, tag="rep", name="rep", bufs=1)
                nc.gpsimd.partition_all_reduce(rep, m1, channels=128,
                                               reduce_op=bass_isa.ReduceOp.max)
                # reshape rows [1, 1024] -> [32(y), 32(x)]: direct sbuf->sbuf
                avgT = sp.tile([32, 38], F32, tag="avgT", name="avgT")
                nc.vector.memset(avgT, 0.0)
                nc.sync.dma_start(out=avgT[:, 3:35], in_=avg_row)
                maxT = sp.tile([32, 38], F32, tag="maxT", name="maxT")
                nc.vector.memset(maxT, 0.0)
                nc.scalar.dma_start(out=maxT[:, 3:35], in_=rep[0:1, :])
                # 7x7 conv as 14 banded matmuls over y, x-shifts on free dim
                psa = ps_a.tile([128, 512], F32, tag="pa", name="pa")
                first = True
                for c2, inp in ((0, avgT), (1, maxT)):
                    for kx in range(7):
                        nc.tensor.matmul(
                            psa[0:32, 0:32],
                            wbd[:, c2 * 7 + kx, :],
                            inp[:, kx:kx + 32],
                            start=first, stop=(c2 == 1 and kx == 6))
                        first = False
                sasb = sp.tile([32, 32], F32, tag="sasb", name="sasb")
                nc.scalar.activation(
                    out=sasb, in_=psa[0:32, 0:32],
                    func=mybir.ActivationFunctionType.Sigmoid)
                sa_row = sp.tile([1, N], F32, tag="sa_row", name="sa_row",
                                 bufs=2)
                nc.sync.dma_start(out=sa_row, in_=sasb)
                sarep = zp.tile([128, N], F32, tag="sarep", name="sarep", bufs=1)
                nc.gpsimd.partition_broadcast(sarep, sa_row, channels=128)
                return sarep

            def c_fin(s, zca, sarep):
                # final: out = relu(z_ca * sa + x). In the s=1 tail, split
                # the four 512-chunks across DVE and Pool so they overlap.
                for cb in range(2):
                    t = op_.tile([128, N], F32, tag="fin", name="fin")
                    o = op_.tile([128, N], F32, tag="fino", name="fino")
                    for ci, (lo, hi) in enumerate(((0, 512), (512, N))):
                        pool_chunk = (s == 0 and ci == 1) or \
                            (s == 1 and cb == 1 and ci == 1)
                        ve = nc.gpsimd if pool_chunk else nc.vector
                        ve.tensor_tensor(
                            out=t[:, lo:hi], in0=zca[cb].bitcast(F32)[:, lo:hi],
                            in1=sarep[:, lo:hi], op=mybir.AluOpType.mult)
                        ve.tensor_tensor(
                            out=t[:, lo:hi], in0=t[:, lo:hi],
                            in1=xt[s][cb].bitcast(F32)[:, lo:hi],
                            op=mybir.AluOpType.add)
                        if s == 0:
                            nc.vector.tensor_scalar(
                                o[:, lo:hi], t[:, lo:hi], 0.0, None,
                                mybir.AluOpType.max)
                        else:
                            nc.scalar.activation(
                                out=o[:, lo:hi], in_=t[:, lo:hi],
                                func=mybir.ActivationFunctionType.Relu)
                        oq = nc.scalar if s == 0 else nc.sync
                        oq.dma_start(out=out[s, cb][:, lo:hi],
                                     in_=o[:, lo:hi])

            def phase_c(s):
                zt, cols = c_conv(s)
                zca, ca = c_chan(s, zt, cols)
                sarep = c_spat(s, zt, zca, ca)
                c_fin(s, zca, sarep)

            phase_a(0)
            phase_b(0, extras=(lambda: a_conv(1), lambda: a_q(1),
                               lambda: a_vt(1), lambda: a_k(1)))
            c0_state = {}

            def x0():
                c0_state["zt"], c0_state["cols"] = c_conv(0)

            def x1():
                c0_state["zca"], c0_state["ca"] = c_chan(
                    0, c0_state["zt"], c0_state["cols"])

            def x2():
                c0_state["sarep"] = c_spat(
                    0, c0_state["zt"], c0_state["zca"], c0_state["ca"])

            def x3():
                c_fin(0, c0_state["zca"], c0_state["sarep"])

            phase_b(1, extras=(x0, x1, x2, x3))
            phase_c(1)

    nc.compile()
    return nc


_NC_CACHE = None


def get_module():
    global _NC_CACHE
    if _NC_CACHE is None:
        _NC_CACHE = build_module()
    return _NC_CACHE


def prep_inputs(x, w1, bn1_g, bn1_b, bn1_m, bn1_v, wq, bq, wk, bk, wv, bv,
                gamma, w2, bn2_g, bn2_b, bn2_m, bn2_v, ca_w1, ca_w2, sa_w):
    """Host-side preprocessing -> per-core in_maps."""
    f64 = np.float64
    s1 = (bn1_g.astype(f64) / np.sqrt(bn1_v.astype(f64) + EPS))
    w1f = (s1[:, None] * w1.astype(f64)).astype(np.float32)
    sh1 = (bn1_b.astype(f64) - bn1_m.astype(f64) * s1).astype(np.float32)
    s2 = (bn2_g.astype(f64) / np.sqrt(bn2_v.astype(f64) + EPS))
    w2f = (s2[:, None] * w2.astype(f64)).astype(np.float32)
    sh2 = (bn2_b.astype(f64) - bn2_m.astype(f64) * s2).astype(np.float32)
    g = float(gamma[0])
    wvg = (wv.astype(f64) * g).astype(np.float32)
    bvg = (bv.astype(f64) * g).astype(np.float32)

    def lhsT(w):  # [O, C] -> [2, 128, O] kb-blocked transpose, tf32
        return tf32_round(np.ascontiguousarray(
            w.T.reshape(2, 128, C)))

    # q/k out-channel permutation for DoubleRow scores:
    # free position sub*128 + h*32 + dlo <- channel h*64 + sub*32 + dlo
    perm = np.empty(C, np.int64)
    for h_ in range(NH):
        for sub in range(2):
            for dlo in range(32):
                perm[sub * 128 + h_ * 32 + dlo] = h_ * 64 + sub * 32 + dlo
    # wall[p, j, kb, c]: stationary weights, kb-blocked transpose
    wall_np = np.stack([lhsT(w1f), lhsT(wq[perm]), lhsT(wk[perm]),
                        lhsT(wvg), lhsT(w2f)], axis=0)  # [5, 2, 128, C]
    wall_np = np.ascontiguousarray(wall_np.transpose(2, 0, 1, 3))
    sm = np.zeros((128, 10), np.float32)
    sm[:, 0:2] = sh1.reshape(2, 128).T
    sm[:, 2:4] = bq[perm].reshape(2, 128).T
    sm[:, 4:6] = bk[perm].reshape(2, 128).T
    sm[:, 6:8] = sh2.reshape(2, 128).T
    sm[:, 8] = 1.0
    base = {
        "wall": wall_np,
        "smalls": sm,
        "bv_r": np.ascontiguousarray(bvg.reshape(1, C)),
    }
    # channel attention weights: caw1T [2, 128, 64]
    c1T = ca_w1.T.astype(np.float32)             # [C, R]
    caw1T = np.concatenate([c1T / float(N), c1T], axis=1)  # [C, 2R]
    base["caw1T"] = np.ascontiguousarray(caw1T.reshape(2, 128, 2 * R))
    caw2T = np.ascontiguousarray(ca_w2.T.astype(np.float32))  # [R, C]
    # spatial conv bands: wband[yi, c2*7+kx, yo] = w[c2, yi-yo+3, kx]
    wb = np.zeros((32, 14, 32), np.float32)
    for c2 in range(2):
        for kx in range(7):
            for yo in range(32):
                for ky in range(7):
                    yi = yo + ky - 3
                    if 0 <= yi < 32:
                        v = sa_w[0, c2, ky, kx]
                        if c2 == 0:
                            v = v / float(C)
                        wb[yi, c2 * 7 + kx, yo] = v
    base["cwb"] = np.concatenate([caw2T, wb.reshape(32, 14 * 32)], axis=1)
    import ml_dtypes
    base["ident"] = np.eye(128, dtype=ml_dtypes.bfloat16)

    xrf = tf32_round(x.reshape(B, C, N))
    in_maps = []
    for core in range(NCORES):
        m = dict(base)
        m["xr"] = np.ascontiguousarray(
            xrf[core * SPC:(core + 1) * SPC].reshape(SPC, 2, 128, N))
        in_maps.append(m)
    return in_maps


def kernel(**inputs):
    nc = get_module()
    in_maps = prep_inputs(**inputs)
    res = run_bass_kernel_spmd(nc, in_maps, core_ids=list(range(NCORES)))
    outs = []
    for core in range(NCORES):
        o = res.results[core]["out"]  # [SPC, 2, 128, N]
        outs.append(o.reshape(SPC, C, H, W))
    return np.concatenate(outs, axis=0)


if __name__ == "__main__":
    nc = get_module()
    print("compiled ok")



# revision 89
# speedup vs baseline: 1.0137x; 1.0137x over previous
"""Trainium2 Bass kernel for nn_BottleneckTransformer.

Data-parallel over batch: B=16 samples -> 8 cores x 2 samples.
Per-core pipeline (per sample):
  A: conv1x1+BN1+relu (r), q/k projections, v^T projection (transposed layout)
  B: attention per head: scores^T = k^T q (K=64), exp on ACT, PV matmul with
     ones-augmented v^T (sumexp via extra column), normalize via
     reciprocal_approx_fast + gpsimd partition_broadcast
  C: conv1x1+BN2 (z), CBAM channel attention (PE matvecs + sigmoid),
     CBAM spatial attention (banded-matrix matmul formulation of the 7x7
     conv), residual + relu.

Matmul dtypes: float32r (tf32) for projections/scores (host-rounded inputs),
bf16 for exp_t/vT/PV (error damped by gamma~0.05), fp32 for tiny CBAM mms.
"""
import numpy as np

import concourse.bacc as bacc
import concourse.bass as bass
import concourse.tile as tile
from concourse import mybir, bass_isa
from concourse.bass_utils import run_bass_kernel_spmd

F32 = mybir.dt.float32
F32R = mybir.dt.float32r
BF16 = mybir.dt.bfloat16
FP8 = mybir.dt.float8e4
U8 = mybir.dt.uint8

# Schraudolph-style exp on DVE: bits = round(score*log2e*0.125*8 + EXP_BIAS)
# bitcast uint8 -> fp8e4m3 approximates exp(score*0.125).
EXP_SCALE = 1.4426950408889634
EXP_BIAS = 55.654

B, C, H, W = 16, 256, 32, 32
N = H * W          # 1024
NCORES = 8
SPC = B // NCORES  # samples per core = 2
NH, D = 4, 64      # heads, head dim
R = C // 8         # 32, channel attention bottleneck
EPS = 1e-5


def tf32_round(x):
    """Round fp32 -> tf32 (10-bit mantissa), round-to-nearest-even."""
    xi = np.ascontiguousarray(x, dtype=np.float32).view(np.uint32)
    lsb = (xi >> np.uint32(13)) & np.uint32(1)
    xi = xi + np.uint32(0x0FFF) + lsb
    xi &= np.uint32(0xFFFFE000)
    return xi.view(np.float32)


def build_module():
    nc = bacc.Bacc("TRN2", target_bir_lowering=False, debug=False)

    def din(name, shape, dt=F32):
        return nc.dram_tensor(name, shape, dt, kind="ExternalInput").ap()

    def dout(name, shape, dt=F32):
        return nc.dram_tensor(name, shape, dt, kind="ExternalOutput").ap()

    xr = din("xr", (SPC, 2, 128, N), F32R)        # per-sample x, c-blocks
    # wall[p, j, kb, c]: j = w1f, wq, wk, wv(gamma), w2f; [c,128 -> o cols]
    wall = din("wall", (128, 5, 2, C), F32R)
    # smalls[p, j]: sh1(2), bq(2), bk(2), sh2(2), ones(1), zeros(1)
    smalls = din("smalls", (128, 10), F32R)
    bv_r = din("bv_r", (1, C), F32)               # gamma folded, row
    caw1T = din("caw1T", (2, 128, 2 * R), F32)    # cols 0:32 avg(/1024), 32:64 max
    cwb = din("cwb", (R, C + 14 * 32), F32)       # caw2T ++ wband rows
    ident = din("ident", (128, 128), BF16)        # PE transpose identity

    out = dout("out", (SPC, 2, 128, N), F32)

    with tile.TileContext(nc) as tc:
        with (
            tc.tile_pool(name="wpool", bufs=1) as wp,
            tc.tile_pool(name="xpool", bufs=1) as xp,
            tc.tile_pool(name="rpool", bufs=1) as rp,
            tc.tile_pool(name="qkpool", bufs=1) as qkp,
            tc.tile_pool(name="vpool", bufs=1) as vp,
            tc.tile_pool(name="epool", bufs=14) as ep,
            tc.tile_pool(name="ypool", bufs=1) as yp,
            tc.tile_pool(name="zpool", bufs=2) as zp,
            tc.tile_pool(name="spool", bufs=2) as sp,
            tc.tile_pool(name="opool", bufs=2) as op_,
            tc.tile_pool(name="ps_sc", bufs=2, space="PSUM") as ps_sc,
            tc.tile_pool(name="ps_at", bufs=1, space="PSUM") as ps_at,
            tc.tile_pool(name="ps_a", bufs=2, space="PSUM") as ps_a,
        ):
            # ---- loads: spread across SP/ACT/DVE/Pool DMA queues so the
            # startup is not serialized on one HWDGE ring. conv1's needs
            # (w1, smalls, first x chunks) land first.
            idn = wp.tile([128, 128], BF16, tag="idn", name="idn")
            nc.scalar.dma_start(out=idn, in_=ident)
            wallt = wp.tile([128, 5, 2, C], F32R, tag="wallt", name="wallt")
            w1t, wqt, wkt, wvt, w2t = (wallt[:, j] for j in range(5))
            nc.scalar.dma_start(out=wallt[:, 0:1], in_=wall[:, 0:1])
            smt = wp.tile([128, 10], F32R, tag="smt", name="smt")
            nc.scalar.dma_start(out=smt, in_=smalls)
            sh1, bqc, bkc, sh2 = (smt.bitcast(F32)[:, 2 * j:2 * j + 2]
                                  for j in range(4))
            ones_fr = smt[:, 8:9]
            zero_fr = smt[:, 9:10]
            xt_all = [[xp.tile([128, N], F32R, tag=f"x{si}{cb}", name=f"x{si}{cb}")
                       for cb in range(2)] for si in range(SPC)]
            for nch in range(2):
                for cb in range(2):
                    nc.sync.dma_start(
                        out=xt_all[0][cb][:, nch * 512:(nch + 1) * 512],
                        in_=xr[0, cb][:, nch * 512:(nch + 1) * 512])
            nc.scalar.dma_start(out=wallt[:, 1:3], in_=wall[:, 1:3])
            nc.gpsimd.dma_start(out=wallt[:, 3:5], in_=wall[:, 3:5])
            for cb in range(2):
                nc.sync.dma_start(out=xt_all[1][cb], in_=xr[1, cb])
            cw1 = wp.tile([128, 2, 2 * R], F32, tag="cw1", name="cw1")
            nc.gpsimd.dma_start(out=cw1, in_=caw1T.rearrange("k p c -> p k c"))
            cwbt = wp.tile([R, C + 14 * 32], F32, tag="cwbt", name="cwbt")
            nc.gpsimd.dma_start(out=cwbt, in_=cwb)
            cw2 = cwbt[:, 0:C]
            wbd = cwbt[:, C:].rearrange("p (a b) -> p a b", a=14)
            bvb = wp.tile([128, C], F32, tag="bvb", name="bvb")
            bv_bc = bass.AP(tensor=bv_r.tensor, offset=bv_r.offset,
                            ap=[[0, 128]] + list(bv_r.ap)[1:])
            nc.gpsimd.dma_start(out=bvb, in_=bv_bc)

            # PE p-state warmup: harmless matmuls on the identity tile keep
            # the tensor engine continuously busy until conv1's inputs land,
            # so conv1 runs at full clock instead of ramping.
            wps = ps_a.tile([128, 512], F32, tag="pa", name="pa")
            NWARM = 18
            for i in range(NWARM):
                nc.tensor.matmul(wps[:, 0:128], idn, idn,
                                 start=(i == 0), stop=(i == NWARM - 1))

            xt = [None] * SPC      # [s][cb] f32r input tiles
            rt = [None] * SPC      # relu(conv1) tiles
            qt = [None] * SPC
            kt = [None] * SPC
            vt = [None] * SPC      # vT_aug bf16 [128, mb, 4*65]
            ytmp = [None] * SPC    # attn accum, then y = attn + r (f32r)

            def a_conv(s):
                xt[s] = xt_all[s]
                rt[s] = [rp.tile([128, N], F32R, tag=f"r{s}{ob}", name=f"r{s}{ob}")
                         for ob in range(2)]
                for ob in range(2):
                    for nch in range(2):
                        pa = ps_a.tile([128, 512], F32, tag="pa", name="pa")
                        for kb in range(2):
                            nc.tensor.matmul(
                                pa, w1t[:, kb, ob * 128:(ob + 1) * 128],
                                xt[s][kb][:, nch * 512:(nch + 1) * 512],
                                start=(kb == 0), stop=(kb == 1))
                        if s == 0:
                            # split startup evictions across ACT and DVE
                            nc.scalar.activation(
                                out=rt[s][ob][:, nch * 512:(nch + 1) * 512],
                                in_=pa, bias=sh1[:, ob:ob + 1],
                                func=mybir.ActivationFunctionType.Relu)
                        else:
                            nc.vector.tensor_scalar(
                                rt[s][ob][:, nch * 512:(nch + 1) * 512], pa,
                                sh1[:, ob:ob + 1], 0.0,
                                mybir.AluOpType.add, mybir.AluOpType.max)

            def a_proj(s, dst, wt, bc, on_act=False, flip=0):
                # dst: [128(h,dlo), 2(dhi), N] fp8 for DoubleRow scores.
                # wt cols are host-permuted so out partition = h*32+dlo.
                for sub in range(2):
                    for nch in range(2):
                        pa = ps_a.tile([128, 512], F32, tag="pa", name="pa")
                        for kb in range(2):
                            nc.tensor.matmul(
                                pa, wt[:, kb, sub * 128:(sub + 1) * 128],
                                rt[s][kb][:, nch * 512:(nch + 1) * 512],
                                start=(kb == 0), stop=(kb == 1))
                        if on_act:
                            nc.scalar.activation(
                                out=dst[:, sub, nch * 512:(nch + 1) * 512],
                                in_=pa, bias=bc[:, sub:sub + 1], scale=1.0,
                                func=mybir.ActivationFunctionType.Identity)
                        else:
                            nc.vector.tensor_scalar(
                                dst[:, sub, nch * 512:(nch + 1) * 512], pa,
                                bc[:, sub:sub + 1], 0.0,
                                mybir.AluOpType.add, mybir.AluOpType.add)

            def a_q(s):
                qt[s] = qkp.tile([128, 2, N], FP8, tag=f"q{s}", name=f"q{s}")
                a_proj(s, qt[s], wqt, bqc, on_act=(s == 0), flip=0)

            def a_k(s):
                kt[s] = qkp.tile([128, 2, N], FP8, tag=f"k{s}", name=f"k{s}")
                a_proj(s, kt[s], wkt, bkc, on_act=(s == 0), flip=1)

            def a_vt(s):
                # [128, mb, head, 80]: strides 16-aligned for DoubleRow lhsT
                vt[s] = vp.tile([128, 8, NH, 80], FP8, tag=f"v{s}", name=f"v{s}")
                for mb in range(8):
                    pa = ps_a.tile([128, 512], F32, tag="pa", name="pa")
                    for kb in range(2):
                        nc.tensor.matmul(
                            pa[:, 0:C],
                            rt[s][kb][:, mb * 128:(mb + 1) * 128],
                            wvt[:, kb, :], start=(kb == 0), stop=(kb == 1))
                    nc.vector.tensor_tensor(
                        out=vt[s][:, mb, :, 0:D],
                        in0=pa[:, 0:C].rearrange("p (h d) -> p h d", h=NH),
                        in1=bvb.rearrange("p (h d) -> p h d", h=NH),
                        op=mybir.AluOpType.add)
                nc.vector.memset(vt[s][:, :, :, D:D + 1], 1.0)

            def phase_a(s):
                # v^T is deferred into phase_b (pre hook): scores only need
                # conv1 + q + k, so exp work starts sooner.
                a_conv(s)
                a_q(s)
                a_k(s)

            def phase_b(s, extras=()):
                ytmp[s] = [yp.tile([128, N], F32R, tag=f"yt{s}{pb}", name=f"yt{s}{pb}")
                           for pb in range(2)]
                et_all = [[None] * 4 for _ in range(NH)]

                def emit_se(h, mb):
                    hs = slice(h * 32, (h + 1) * 32)
                    psc = ps_sc.tile([128, 1024], F32, tag="psc", name="psc")
                    for nch in range(2):
                        nc.tensor.matmul(
                            psc[:, nch * 512:(nch + 1) * 512],
                            kt[s][hs, :, mb * 128:(mb + 1) * 128],
                            qt[s][hs, :, nch * 512:(nch + 1) * 512],
                            start=True, stop=True,
                            perf_mode=mybir.MatmulPerfMode.DoubleRow,
                            tile_position=(h * 32, 0))
                    if mb % 2 == 0:
                        et_all[h][mb // 2] = ep.tile(
                            [128, 2, 1024], FP8, tag="et", name="et")
                    e = et_all[h][mb // 2]
                    if mb in (0, 5):
                        # bit-trick exp on DVE to offload the ACT engine
                        nc.vector.tensor_scalar(
                            e[:, mb % 2, :].bitcast(U8), psc,
                            EXP_SCALE, EXP_BIAS,
                            mybir.AluOpType.mult, mybir.AluOpType.add)
                    else:
                        nc.scalar.activation(
                            out=e[:, mb % 2, :], in_=psc,
                            func=mybir.ActivationFunctionType.Exp, scale=0.125)

                def emit_se_pair2(hp, mb):
                    # heads 2hp (rows 0-63) and 2hp+1 (rows 64-127):
                    # alternate MMs so adjacent instructions use different
                    # PE row groups and overlap on hardware
                    pb = hp
                    pscs = []
                    for j in range(2):
                        pscs.append(ps_sc.tile([128, 1024], F32, tag="psc",
                                               name="psc"))
                    for nch in range(2):
                        for j in range(2):
                            dsl = slice(j * 64, j * 64 + 64)
                            nc.tensor.matmul(
                                pscs[j][:, nch * 512:(nch + 1) * 512],
                                kt[s][pb][dsl, mb * 128:(mb + 1) * 128],
                                qt[s][pb][dsl, nch * 512:(nch + 1) * 512],
                                start=True, stop=True)
                    for j in range(2):
                        e = ep.tile([128, 1024], FP8, tag="et", name="et")
                        nc.scalar.activation(
                            out=e, in_=pscs[j],
                            func=mybir.ActivationFunctionType.Exp, scale=0.125)
                        et_all[2 * hp + j][mb] = e

                ytp = [None, None]

                def emit_pv(h):
                    # PV transposed: patT[n, d] per 128-n block; sumexp in
                    # col D; normalize per-partition; PE-transpose back.
                    pb, hh = h // 2, h % 2
                    et = et_all[h]
                    yT = sp.tile([128, 8, D], BF16, tag="yT", name="yT",
                                 bufs=2)
                    for half in range(2):
                        patT = ps_at.tile([128, 512], F32, tag="pat",
                                          name="pat")
                        for blk4 in range(4):
                            blk = half * 4 + blk4
                            for pr in range(4):
                                nc.tensor.matmul(
                                    patT[:, blk4 * 128:blk4 * 128 + D + 1],
                                    et[pr][:, :, blk * 128:(blk + 1) * 128],
                                    vt[s][:, 2 * pr:2 * pr + 2, h, 0:D + 1],
                                    start=(pr == 0), stop=(pr == 3),
                                    perf_mode=mybir.MatmulPerfMode.DoubleRow)
                        pv = patT.rearrange("p (b c) -> p b c", b=4)
                        rt4 = sp.tile([128, 4], F32, tag="rt4", name="rt4",
                                      bufs=4)
                        nc.vector.reciprocal_approx_fast(
                            out=rt4, in_=pv[:, :, D])
                        rb = bass.AP(tensor=rt4.tensor, offset=rt4.offset,
                                     ap=list(rt4.ap) + [[0, D]])
                        nc.vector.tensor_tensor(
                            out=yT[:, half * 4:(half + 1) * 4, :],
                            in0=pv[:, :, 0:D], in1=rb,
                            op=mybir.AluOpType.mult)
                    if hh == 0:
                        ytp[pb] = ps_at.tile([128, N], BF16, tag="ytp",
                                             name="ytp", bufs=1)
                    for blk in range(8):
                        nc.tensor.transpose(
                            out=ytp[pb][hh * 64:hh * 64 + 64,
                                        blk * 128:(blk + 1) * 128],
                            in_=yT[:, blk, :], identity=idn)

                PF = 8
                for h in range(NH):
                    for mb in (range(PF, 8) if h > 0 else range(8)):
                        emit_se(h, mb)
                    if h == 0 and vt[s] is None:
                        a_vt(s)  # overlap v^T with the first exp batch
                    if h + 1 < NH:
                        for mb in range(PF):
                            emit_se(h + 1, mb)
                    emit_pv(h)
                    if h % 2 == 1:
                        # y = attn + r (rounded to f32r for conv2)
                        pb = h // 2
                        nc.vector.tensor_tensor(
                            out=ytmp[s][pb],
                            in0=ytp[pb],
                            in1=rt[s][pb].bitcast(F32),
                            op=mybir.AluOpType.add)
                    if extras and h < len(extras):
                        extras[h]()

            def c_conv(s):
                # conv2 + bn2 -> z (fp32), with per-channel sums for CBAM avg.
                # Per-chunk sum accum + max reduce so the pools pipeline with
                # the matmuls instead of serializing after the full tile.
                zt = [zp.tile([128, N], F32R, tag=f"z{ob}", name=f"z{ob}")
                      for ob in range(2)]
                cols = [zp.tile([128, 4], F32, tag=f"cols{ob}", name=f"cols{ob}")
                        for ob in range(2)]
                for ob in range(2):
                    for nch in range(2):
                        pa = ps_a.tile([128, 512], F32, tag="pa", name="pa")
                        for kb in range(2):
                            nc.tensor.matmul(
                                pa, w2t[:, kb, ob * 128:(ob + 1) * 128],
                                ytmp[s][kb][:, nch * 512:(nch + 1) * 512],
                                start=(kb == 0), stop=(kb == 1))
                        if s == 1 and ob == 0:
                            nc.scalar.activation(
                                out=zt[ob][:, nch * 512:(nch + 1) * 512],
                                in_=pa, bias=sh2[:, ob:ob + 1],
                                func=mybir.ActivationFunctionType.Identity,
                                accum_out=cols[ob][:, nch:nch + 1])
                        else:
                            nc.vector.tensor_scalar(
                                zt[ob][:, nch * 512:(nch + 1) * 512], pa,
                                sh2[:, ob:ob + 1], 0.0,
                                mybir.AluOpType.add, mybir.AluOpType.add,
                                accum_out=cols[ob][:, nch:nch + 1])
                        nc.vector.tensor_reduce(
                            out=cols[ob][:, 2 + nch:3 + nch],
                            in_=zt[ob][:, nch * 512:(nch + 1) * 512],
                            op=mybir.AluOpType.max,
                            axis=mybir.AxisListType.X)
                    # combine chunk sums into col 0, chunk maxes into col 1
                    nc.vector.tensor_tensor(
                        out=cols[ob][:, 0:1], in0=cols[ob][:, 0:1],
                        in1=cols[ob][:, 1:2], op=mybir.AluOpType.add)
                    nc.vector.tensor_tensor(
                        out=cols[ob][:, 1:2], in0=cols[ob][:, 2:3],
                        in1=cols[ob][:, 3:4], op=mybir.AluOpType.max)
                return zt, cols

            def c_chan(s, zt, cols):
                # channel attention: h = relu(W1a@sum) + relu(W1m@max),
                # ca = sig(W2@h)
                ph = ps_a.tile([128, 512], F32, tag="pa", name="pa")
                for j in range(2):
                    for kb in range(2):
                        nc.tensor.matmul(ph[0:R, j:j + 1],
                                         cw1[:, kb, j * R:(j + 1) * R],
                                         cols[kb][:, j:j + 1],
                                         start=(kb == 0), stop=(kb == 1))
                hsb = sp.tile([R, 3], F32, tag="hsb", name="hsb")
                nc.vector.tensor_scalar(hsb[:, 0:2], ph[0:R, 0:2], 0.0, None,
                                        mybir.AluOpType.max)
                nc.vector.tensor_tensor(out=hsb[:, 2:3], in0=hsb[:, 0:1],
                                        in1=hsb[:, 1:2],
                                        op=mybir.AluOpType.add)
                ca = [sp.tile([128, 1], F32R, tag=f"ca{ob}", name=f"ca{ob}")
                      for ob in range(2)]
                pc = ps_a.tile([128, 512], F32, tag="pa", name="pa")
                for ob in range(2):
                    nc.tensor.matmul(pc[:, ob:ob + 1],
                                     cw2[:, ob * 128:(ob + 1) * 128],
                                     hsb[:, 2:3], start=True, stop=True)
                for ob in range(2):
                    nc.scalar.activation(
                        out=ca[ob], in_=pc[:, ob:ob + 1],
                        func=mybir.ActivationFunctionType.Sigmoid)
                # apply channel attention -> z_ca (f32r for the ones-matmul).
                # ob=0 on DVE, ob=1 on ACT (Copy with per-partition scale AP)
                # so the two run in parallel.
                zca = [zp.tile([128, N], F32R, tag=f"zca{ob}", name=f"zca{ob}")
                       for ob in range(2)]
                nc.vector.tensor_scalar_mul(zca[0], zt[0],
                                            ca[0].bitcast(F32))
                nc.scalar.activation(
                    out=zca[1], in_=zt[1],
                    func=mybir.ActivationFunctionType.Copy,
                    scale=ca[1].bitcast(F32))
                return zca, ca

            def c_spat(s, zt, zca, ca):
                # spatial sum (avg path): ca^T @ z, so it does not wait on
                # the zca tiles
                avg_row = sp.tile([1, N], F32, tag="avg_row", name="avg_row", bufs=1)
                for nch in range(2):
                    psr = ps_a.tile([128, 512], F32, tag="pa", name="pa")
                    for kb in range(2):
                        nc.tensor.matmul(
                            psr[0:1, :],
                            ca[kb], zt[kb][:, nch * 512:(nch + 1) * 512],
                            start=(kb == 0), stop=(kb == 1))
                    nc.scalar.copy(
                        avg_row[:, nch * 512:(nch + 1) * 512], psr[0:1, :])
                # spatial max via TT max + gpsimd cross-partition reduce
                m1 = zp.tile([128, N], F32, tag="m1", name="m1", bufs=1)
                nc.vector.tensor_tensor(out=m1, in0=zca[0].bitcast(F32),
                                        in1=zca[1].bitcast(F32),
                                        op=mybir.AluOpType.max)
                rep = zp.tile([128, N], F32, tag="rep", name="rep", bufs=1)
                nc.gpsimd.partition_all_reduce(rep, m1, channels=128,
                                               reduce_op=bass_isa.ReduceOp.max)
                # reshape rows [1, 1024] -> [32(y), 32(x)]: direct sbuf->sbuf
                avgT = sp.tile([32, 38], F32, tag="avgT", name="avgT")
                nc.vector.memset(avgT, 0.0)
                nc.sync.dma_start(out=avgT[:, 3:35], in_=avg_row)
                maxT = sp.tile([32, 38], F32, tag="maxT", name="maxT")
                nc.vector.memset(maxT, 0.0)
                nc.scalar.dma_start(out=maxT[:, 3:35], in_=rep[0:1, :])
                # 7x7 conv as 14 banded matmuls over y, x-shifts on free dim
                psa = ps_a.tile([128, 512], F32, tag="pa", name="pa")
                first = True
                for c2, inp in ((0, avgT), (1, maxT)):
                    for kx in range(7):
                        nc.tensor.matmul(
                            psa[0:32, 0:32],
                            wbd[:, c2 * 7 + kx, :],
                            inp[:, kx:kx + 32],
                            start=first, stop=(c2 == 1 and kx == 6))
                        first = False
                sasb = sp.tile([32, 32], F32, tag="sasb", name="sasb")
                nc.scalar.activation(
                    out=sasb, in_=psa[0:32, 0:32],
                    func=mybir.ActivationFunctionType.Sigmoid)
                sa_row = sp.tile([1, N], F32, tag="sa_row", name="sa_row",
                                 bufs=2)
                nc.sync.dma_start(out=sa_row, in_=sasb)
                sarep = zp.tile([128, N], F32, tag="sarep", name="sarep", bufs=1)
                nc.gpsimd.partition_broadcast(sarep, sa_row, channels=128)
                return sarep

            def c_fin(s, zca, sarep):
                # final: out = relu(z_ca * sa + x). In the s=1 tail, split
                # the four 512-chunks across DVE and Pool so they overlap.
                for cb in range(2):
                    t = op_.tile([128, N], F32, tag="fin", name="fin")
                    o = op_.tile([128, N], F32, tag="fino", name="fino")
                    for ci, (lo, hi) in enumerate(((0, 512), (512, N))):
                        pool_chunk = (s == 0 and ci == 1) or \
                            (s == 1 and cb == 1 and ci == 1)
                        ve = nc.gpsimd if pool_chunk else nc.vector
                        ve.tensor_tensor(
                            out=t[:, lo:hi], in0=zca[cb].bitcast(F32)[:, lo:hi],
                            in1=sarep[:, lo:hi], op=mybir.AluOpType.mult)
                        ve.tensor_tensor(
                            out=t[:, lo:hi], in0=t[:, lo:hi],
                            in1=xt[s][cb].bitcast(F32)[:, lo:hi],
                            op=mybir.AluOpType.add)
                        if s == 0:
                            nc.vector.tensor_scalar(
                                o[:, lo:hi], t[:, lo:hi], 0.0, None,
                                mybir.AluOpType.max)
                        else:
                            nc.scalar.activation(
                                out=o[:, lo:hi], in_=t[:, lo:hi],
                                func=mybir.ActivationFunctionType.Relu)
                        oq = nc.scalar if s == 0 else nc.sync
                        oq.dma_start(out=out[s, cb][:, lo:hi],
                                     in_=o[:, lo:hi])

            def phase_c(s):
                zt, cols = c_conv(s)
                zca, ca = c_chan(s, zt, cols)
                sarep = c_spat(s, zt, zca, ca)
                c_fin(s, zca, sarep)

            phase_a(0)
            phase_b(0, extras=(lambda: a_conv(1), lambda: a_q(1),
                               lambda: a_vt(1), lambda: a_k(1)))
            c0_state = {}

            def x0():
                c0_state["zt"], c0_state["cols"] = c_conv(0)

            def x1():
                c0_state["zca"], c0_state["ca"] = c_chan(
                    0, c0_state["zt"], c0_state["cols"])

            def x2():
                c0_state["sarep"] = c_spat(
                    0, c0_state["zt"], c0_state["zca"], c0_state["ca"])

            def x3():
                c_fin(0, c0_state["zca"], c0_state["sarep"])

            phase_b(1, extras=(x0, x1, x2, x3))
            phase_c(1)

    nc.compile()
    return nc


_NC_CACHE = None


def get_module():
    global _NC_CACHE
    if _NC_CACHE is None:
        _NC_CACHE = build_module()
    return _NC_CACHE


def prep_inputs(x, w1, bn1_g, bn1_b, bn1_m, bn1_v, wq, bq, wk, bk, wv, bv,
                gamma, w2, bn2_g, bn2_b, bn2_m, bn2_v, ca_w1, ca_w2, sa_w):
    """Host-side preprocessing -> per-core in_maps."""
    f64 = np.float64
    s1 = (bn1_g.astype(f64) / np.sqrt(bn1_v.astype(f64) + EPS))
    w1f = (s1[:, None] * w1.astype(f64)).astype(np.float32)
    sh1 = (bn1_b.astype(f64) - bn1_m.astype(f64) * s1).astype(np.float32)
    s2 = (bn2_g.astype(f64) / np.sqrt(bn2_v.astype(f64) + EPS))
    w2f = (s2[:, None] * w2.astype(f64)).astype(np.float32)
    sh2 = (bn2_b.astype(f64) - bn2_m.astype(f64) * s2).astype(np.float32)
    g = float(gamma[0])
    wvg = (wv.astype(f64) * g).astype(np.float32)
    bvg = (bv.astype(f64) * g).astype(np.float32)

    def lhsT(w):  # [O, C] -> [2, 128, O] kb-blocked transpose, tf32
        return tf32_round(np.ascontiguousarray(
            w.T.reshape(2, 128, C)))

    # q/k out-channel permutation for DoubleRow scores:
    # free position sub*128 + h*32 + dlo <- channel h*64 + sub*32 + dlo
    perm = np.empty(C, np.int64)
    for h_ in range(NH):
        for sub in range(2):
            for dlo in range(32):
                perm[sub * 128 + h_ * 32 + dlo] = h_ * 64 + sub * 32 + dlo
    # wall[p, j, kb, c]: stationary weights, kb-blocked transpose
    wall_np = np.stack([lhsT(w1f), lhsT(wq[perm]), lhsT(wk[perm]),
                        lhsT(wvg), lhsT(w2f)], axis=0)  # [5, 2, 128, C]
    wall_np = np.ascontiguousarray(wall_np.transpose(2, 0, 1, 3))
    sm = np.zeros((128, 10), np.float32)
    sm[:, 0:2] = sh1.reshape(2, 128).T
    sm[:, 2:4] = bq[perm].reshape(2, 128).T
    sm[:, 4:6] = bk[perm].reshape(2, 128).T
    sm[:, 6:8] = sh2.reshape(2, 128).T
    sm[:, 8] = 1.0
    base = {
        "wall": wall_np,
        "smalls": sm,
        "bv_r": np.ascontiguousarray(bvg.reshape(1, C)),
    }
    # channel attention weights: caw1T [2, 128, 64]
    c1T = ca_w1.T.astype(np.float32)             # [C, R]
    caw1T = np.concatenate([c1T / float(N), c1T], axis=1)  # [C, 2R]
    base["caw1T"] = np.ascontiguousarray(caw1T.reshape(2, 128, 2 * R))
    caw2T = np.ascontiguousarray(ca_w2.T.astype(np.float32))  # [R, C]
    # spatial conv bands: wband[yi, c2*7+kx, yo] = w[c2, yi-yo+3, kx]
    wb = np.zeros((32, 14, 32), np.float32)
    for c2 in range(2):
        for kx in range(7):
            for yo in range(32):
                for ky in range(7):
                    yi = yo + ky - 3
                    if 0 <= yi < 32:
                        v = sa_w[0, c2, ky, kx]
                        if c2 == 0:
                            v = v / float(C)
                        wb[yi, c2 * 7 + kx, yo] = v
    base["cwb"] = np.concatenate([caw2T, wb.reshape(32, 14 * 32)], axis=1)
    import ml_dtypes
    base["ident"] = np.eye(128, dtype=ml_dtypes.bfloat16)

    xrf = tf32_round(x.reshape(B, C, N))
    in_maps = []
    for core in range(NCORES):
        m = dict(base)
        m["xr"] = np.ascontiguousarray(
            xrf[core * SPC:(core + 1) * SPC].reshape(SPC, 2, 128, N))
        in_maps.append(m)
    return in_maps


def kernel(**inputs):
    nc = get_module()
    in_maps = prep_inputs(**inputs)
    res = run_bass_kernel_spmd(nc, in_maps, core_ids=list(range(NCORES)))
    outs = []
    for core in range(NCORES):
        o = res.results[core]["out"]  # [SPC, 2, 128, N]
        outs.append(o.reshape(SPC, C, H, W))
    return np.concatenate(outs, axis=0)


if __name__ == "__main__":
    nc = get_module()
    print("compiled ok")



# revision 90
# speedup vs baseline: 1.0185x; 1.0048x over previous
"""Trainium2 Bass kernel for nn_BottleneckTransformer.

Data-parallel over batch: B=16 samples -> 8 cores x 2 samples.
Per-core pipeline (per sample):
  A: conv1x1+BN1+relu (r), q/k projections, v^T projection (transposed layout)
  B: attention per head: scores^T = k^T q (K=64), exp on ACT, PV matmul with
     ones-augmented v^T (sumexp via extra column), normalize via
     reciprocal_approx_fast + gpsimd partition_broadcast
  C: conv1x1+BN2 (z), CBAM channel attention (PE matvecs + sigmoid),
     CBAM spatial attention (banded-matrix matmul formulation of the 7x7
     conv), residual + relu.

Matmul dtypes: float32r (tf32) for projections/scores (host-rounded inputs),
bf16 for exp_t/vT/PV (error damped by gamma~0.05), fp32 for tiny CBAM mms.
"""
import numpy as np

import concourse.bacc as bacc
import concourse.bass as bass
import concourse.tile as tile
from concourse import mybir, bass_isa
from concourse.bass_utils import run_bass_kernel_spmd

F32 = mybir.dt.float32
F32R = mybir.dt.float32r
BF16 = mybir.dt.bfloat16
FP8 = mybir.dt.float8e4
U8 = mybir.dt.uint8

# Schraudolph-style exp on DVE: bits = round(score*log2e*0.125*8 + EXP_BIAS)
# bitcast uint8 -> fp8e4m3 approximates exp(score*0.125).
EXP_SCALE = 1.4426950408889634
EXP_BIAS = 55.654

B, C, H, W = 16, 256, 32, 32
N = H * W          # 1024
NCORES = 8
SPC = B // NCORES  # samples per core = 2
NH, D = 4, 64      # heads, head dim
R = C // 8         # 32, channel attention bottleneck
EPS = 1e-5


def tf32_round(x):
    """Round fp32 -> tf32 (10-bit mantissa), round-to-nearest-even."""
    xi = np.ascontiguousarray(x, dtype=np.float32).view(np.uint32)
    lsb = (xi >> np.uint32(13)) & np.uint32(1)
    xi = xi + np.uint32(0x0FFF) + lsb
    xi &= np.uint32(0xFFFFE000)
    return xi.view(np.float32)


def build_module():
    nc = bacc.Bacc("TRN2", target_bir_lowering=False, debug=False)

    def din(name, shape, dt=F32):
        return nc.dram_tensor(name, shape, dt, kind="ExternalInput").ap()

    def dout(name, shape, dt=F32):
        return nc.dram_tensor(name, shape, dt, kind="ExternalOutput").ap()

    xr = din("xr", (SPC, 2, 128, N), F32R)        # per-sample x, c-blocks
    # wall[p, j, kb, c]: j = w1f, wq, wk, wv(gamma), w2f; [c,128 -> o cols]
    wall = din("wall", (128, 5, 2, C), F32R)
    # smalls[p, j]: sh1(2), bq(2), bk(2), sh2(2), ones(1), zeros(1)
    smalls = din("smalls", (128, 10), F32R)
    bv_r = din("bv_r", (1, C), F32)               # gamma folded, row
    caw1T = din("caw1T", (2, 128, 2 * R), F32)    # cols 0:32 avg(/1024), 32:64 max
    cwb = din("cwb", (R, C + 14 * 32), F32)       # caw2T ++ wband rows
    ident = din("ident", (128, 128), BF16)        # PE transpose identity

    out = dout("out", (SPC, 2, 128, N), F32)

    with tile.TileContext(nc) as tc:
        with (
            tc.tile_pool(name="wpool", bufs=1) as wp,
            tc.tile_pool(name="xpool", bufs=1) as xp,
            tc.tile_pool(name="rpool", bufs=1) as rp,
            tc.tile_pool(name="qkpool", bufs=1) as qkp,
            tc.tile_pool(name="vpool", bufs=1) as vp,
            tc.tile_pool(name="epool", bufs=14) as ep,
            tc.tile_pool(name="ypool", bufs=1) as yp,
            tc.tile_pool(name="zpool", bufs=2) as zp,
            tc.tile_pool(name="spool", bufs=2) as sp,
            tc.tile_pool(name="opool", bufs=2) as op_,
            tc.tile_pool(name="ps_sc", bufs=2, space="PSUM") as ps_sc,
            tc.tile_pool(name="ps_at", bufs=1, space="PSUM") as ps_at,
            tc.tile_pool(name="ps_a", bufs=2, space="PSUM") as ps_a,
        ):
            # ---- loads: spread across SP/ACT/DVE/Pool DMA queues so the
            # startup is not serialized on one HWDGE ring. conv1's needs
            # (w1, smalls, first x chunks) land first.
            idn = wp.tile([128, 128], BF16, tag="idn", name="idn")
            nc.scalar.dma_start(out=idn, in_=ident)
            wallt = wp.tile([128, 5, 2, C], F32R, tag="wallt", name="wallt")
            w1t, wqt, wkt, wvt, w2t = (wallt[:, j] for j in range(5))
            nc.scalar.dma_start(out=wallt[:, 0:1], in_=wall[:, 0:1])
            smt = wp.tile([128, 10], F32R, tag="smt", name="smt")
            nc.scalar.dma_start(out=smt, in_=smalls)
            sh1, bqc, bkc, sh2 = (smt.bitcast(F32)[:, 2 * j:2 * j + 2]
                                  for j in range(4))
            ones_fr = smt[:, 8:9]
            zero_fr = smt[:, 9:10]
            xt_all = [[xp.tile([128, N], F32R, tag=f"x{si}{cb}", name=f"x{si}{cb}")
                       for cb in range(2)] for si in range(SPC)]
            for nch in range(2):
                for cb in range(2):
                    nc.sync.dma_start(
                        out=xt_all[0][cb][:, nch * 512:(nch + 1) * 512],
                        in_=xr[0, cb][:, nch * 512:(nch + 1) * 512])
            nc.scalar.dma_start(out=wallt[:, 1:3], in_=wall[:, 1:3])
            nc.gpsimd.dma_start(out=wallt[:, 3:5], in_=wall[:, 3:5])
            for cb in range(2):
                nc.sync.dma_start(out=xt_all[1][cb], in_=xr[1, cb])
            cw1 = wp.tile([128, 2, 2 * R], F32, tag="cw1", name="cw1")
            nc.gpsimd.dma_start(out=cw1, in_=caw1T.rearrange("k p c -> p k c"))
            cwbt = wp.tile([R, C + 14 * 32], F32, tag="cwbt", name="cwbt")
            nc.gpsimd.dma_start(out=cwbt, in_=cwb)
            cw2 = cwbt[:, 0:C]
            wbd = cwbt[:, C:].rearrange("p (a b) -> p a b", a=14)
            bvb = wp.tile([128, C], F32, tag="bvb", name="bvb")
            bv_bc = bass.AP(tensor=bv_r.tensor, offset=bv_r.offset,
                            ap=[[0, 128]] + list(bv_r.ap)[1:])
            nc.gpsimd.dma_start(out=bvb, in_=bv_bc)

            # PE p-state warmup: harmless matmuls on the identity tile keep
            # the tensor engine continuously busy until conv1's inputs land,
            # so conv1 runs at full clock instead of ramping.
            wps = ps_a.tile([128, 512], F32, tag="pa", name="pa")
            NWARM = 14
            for i in range(NWARM):
                nc.tensor.matmul(wps[:, 0:128], idn, idn,
                                 start=(i == 0), stop=(i == NWARM - 1))

            xt = [None] * SPC      # [s][cb] f32r input tiles
            rt = [None] * SPC      # relu(conv1) tiles
            qt = [None] * SPC
            kt = [None] * SPC
            vt = [None] * SPC      # vT_aug bf16 [128, mb, 4*65]
            ytmp = [None] * SPC    # attn accum, then y = attn + r (f32r)

            def a_conv(s):
                xt[s] = xt_all[s]
                rt[s] = [rp.tile([128, N], F32R, tag=f"r{s}{ob}", name=f"r{s}{ob}")
                         for ob in range(2)]
                for ob in range(2):
                    for nch in range(2):
                        pa = ps_a.tile([128, 512], F32, tag="pa", name="pa")
                        for kb in range(2):
                            nc.tensor.matmul(
                                pa, w1t[:, kb, ob * 128:(ob + 1) * 128],
                                xt[s][kb][:, nch * 512:(nch + 1) * 512],
                                start=(kb == 0), stop=(kb == 1))
                        if s == 0:
                            # split startup evictions across ACT and DVE
                            nc.scalar.activation(
                                out=rt[s][ob][:, nch * 512:(nch + 1) * 512],
                                in_=pa, bias=sh1[:, ob:ob + 1],
                                func=mybir.ActivationFunctionType.Relu)
                        else:
                            nc.vector.tensor_scalar(
                                rt[s][ob][:, nch * 512:(nch + 1) * 512], pa,
                                sh1[:, ob:ob + 1], 0.0,
                                mybir.AluOpType.add, mybir.AluOpType.max)

            def a_proj(s, dst, wt, bc, on_act=False, flip=0):
                # dst: [128(h,dlo), 2(dhi), N] fp8 for DoubleRow scores.
                # wt cols are host-permuted so out partition = h*32+dlo.
                for sub in range(2):
                    for nch in range(2):
                        pa = ps_a.tile([128, 512], F32, tag="pa", name="pa")
                        for kb in range(2):
                            nc.tensor.matmul(
                                pa, wt[:, kb, sub * 128:(sub + 1) * 128],
                                rt[s][kb][:, nch * 512:(nch + 1) * 512],
                                start=(kb == 0), stop=(kb == 1))
                        if on_act:
                            nc.scalar.activation(
                                out=dst[:, sub, nch * 512:(nch + 1) * 512],
                                in_=pa, bias=bc[:, sub:sub + 1], scale=1.0,
                                func=mybir.ActivationFunctionType.Identity)
                        else:
                            nc.vector.tensor_scalar(
                                dst[:, sub, nch * 512:(nch + 1) * 512], pa,
                                bc[:, sub:sub + 1], 0.0,
                                mybir.AluOpType.add, mybir.AluOpType.add)

            def a_q(s):
                qt[s] = qkp.tile([128, 2, N], FP8, tag=f"q{s}", name=f"q{s}")
                a_proj(s, qt[s], wqt, bqc, on_act=(s == 0), flip=0)

            def a_k(s):
                kt[s] = qkp.tile([128, 2, N], FP8, tag=f"k{s}", name=f"k{s}")
                a_proj(s, kt[s], wkt, bkc, on_act=(s == 0), flip=1)

            def a_vt(s):
                # [128, mb, head, 80]: strides 16-aligned for DoubleRow lhsT
                vt[s] = vp.tile([128, 8, NH, 80], FP8, tag=f"v{s}", name=f"v{s}")
                for mb in range(8):
                    pa = ps_a.tile([128, 512], F32, tag="pa", name="pa")
                    for kb in range(2):
                        nc.tensor.matmul(
                            pa[:, 0:C],
                            rt[s][kb][:, mb * 128:(mb + 1) * 128],
                            wvt[:, kb, :], start=(kb == 0), stop=(kb == 1))
                    nc.vector.tensor_tensor(
                        out=vt[s][:, mb, :, 0:D],
                        in0=pa[:, 0:C].rearrange("p (h d) -> p h d", h=NH),
                        in1=bvb.rearrange("p (h d) -> p h d", h=NH),
                        op=mybir.AluOpType.add)
                nc.vector.memset(vt[s][:, :, :, D:D + 1], 1.0)

            def phase_a(s):
                # v^T is deferred into phase_b (pre hook): scores only need
                # conv1 + q + k, so exp work starts sooner.
                a_conv(s)
                a_q(s)
                a_k(s)

            def phase_b(s, extras=()):
                ytmp[s] = [yp.tile([128, N], F32R, tag=f"yt{s}{pb}", name=f"yt{s}{pb}")
                           for pb in range(2)]
                et_all = [[None] * 4 for _ in range(NH)]

                def emit_se(h, mb):
                    hs = slice(h * 32, (h + 1) * 32)
                    psc = ps_sc.tile([128, 1024], F32, tag="psc", name="psc")
                    for nch in range(2):
                        nc.tensor.matmul(
                            psc[:, nch * 512:(nch + 1) * 512],
                            kt[s][hs, :, mb * 128:(mb + 1) * 128],
                            qt[s][hs, :, nch * 512:(nch + 1) * 512],
                            start=True, stop=True,
                            perf_mode=mybir.MatmulPerfMode.DoubleRow,
                            tile_position=(h * 32, 0))
                    if mb % 2 == 0:
                        et_all[h][mb // 2] = ep.tile(
                            [128, 2, 1024], FP8, tag="et", name="et")
                    e = et_all[h][mb // 2]
                    if mb in (0, 5):
                        # bit-trick exp on DVE to offload the ACT engine
                        nc.vector.tensor_scalar(
                            e[:, mb % 2, :].bitcast(U8), psc,
                            EXP_SCALE, EXP_BIAS,
                            mybir.AluOpType.mult, mybir.AluOpType.add)
                    else:
                        nc.scalar.activation(
                            out=e[:, mb % 2, :], in_=psc,
                            func=mybir.ActivationFunctionType.Exp, scale=0.125)

                def emit_se_pair2(hp, mb):
                    # heads 2hp (rows 0-63) and 2hp+1 (rows 64-127):
                    # alternate MMs so adjacent instructions use different
                    # PE row groups and overlap on hardware
                    pb = hp
                    pscs = []
                    for j in range(2):
                        pscs.append(ps_sc.tile([128, 1024], F32, tag="psc",
                                               name="psc"))
                    for nch in range(2):
                        for j in range(2):
                            dsl = slice(j * 64, j * 64 + 64)
                            nc.tensor.matmul(
                                pscs[j][:, nch * 512:(nch + 1) * 512],
                                kt[s][pb][dsl, mb * 128:(mb + 1) * 128],
                                qt[s][pb][dsl, nch * 512:(nch + 1) * 512],
                                start=True, stop=True)
                    for j in range(2):
                        e = ep.tile([128, 1024], FP8, tag="et", name="et")
                        nc.scalar.activation(
                            out=e, in_=pscs[j],
                            func=mybir.ActivationFunctionType.Exp, scale=0.125)
                        et_all[2 * hp + j][mb] = e

                ytp = [None, None]

                def emit_pv(h):
                    # PV transposed: patT[n, d] per 128-n block; sumexp in
                    # col D; normalize per-partition; PE-transpose back.
                    pb, hh = h // 2, h % 2
                    et = et_all[h]
                    yT = sp.tile([128, 8, D], BF16, tag="yT", name="yT",
                                 bufs=2)
                    for half in range(2):
                        patT = ps_at.tile([128, 512], F32, tag="pat",
                                          name="pat")
                        for blk4 in range(4):
                            blk = half * 4 + blk4
                            for pr in range(4):
                                nc.tensor.matmul(
                                    patT[:, blk4 * 128:blk4 * 128 + D + 1],
                                    et[pr][:, :, blk * 128:(blk + 1) * 128],
                                    vt[s][:, 2 * pr:2 * pr + 2, h, 0:D + 1],
                                    start=(pr == 0), stop=(pr == 3),
                                    perf_mode=mybir.MatmulPerfMode.DoubleRow)
                        pv = patT.rearrange("p (b c) -> p b c", b=4)
                        rt4 = sp.tile([128, 4], F32, tag="rt4", name="rt4",
                                      bufs=4)
                        nc.vector.reciprocal_approx_fast(
                            out=rt4, in_=pv[:, :, D])
                        rb = bass.AP(tensor=rt4.tensor, offset=rt4.offset,
                                     ap=list(rt4.ap) + [[0, D]])
                        nc.vector.tensor_tensor(
                            out=yT[:, half * 4:(half + 1) * 4, :],
                            in0=pv[:, :, 0:D], in1=rb,
                            op=mybir.AluOpType.mult)
                    if hh == 0:
                        ytp[pb] = ps_at.tile([128, N], BF16, tag="ytp",
                                             name="ytp", bufs=1)
                    for blk in range(8):
                        nc.tensor.transpose(
                            out=ytp[pb][hh * 64:hh * 64 + 64,
                                        blk * 128:(blk + 1) * 128],
                            in_=yT[:, blk, :], identity=idn)

                PF = 8
                for h in range(NH):
                    for mb in (range(PF, 8) if h > 0 else range(8)):
                        emit_se(h, mb)
                    if h == 0 and vt[s] is None:
                        a_vt(s)  # overlap v^T with the first exp batch
                    if h + 1 < NH:
                        for mb in range(PF):
                            emit_se(h + 1, mb)
                    emit_pv(h)
                    if h % 2 == 1:
                        # y = attn + r (rounded to f32r for conv2)
                        pb = h // 2
                        nc.vector.tensor_tensor(
                            out=ytmp[s][pb],
                            in0=ytp[pb],
                            in1=rt[s][pb].bitcast(F32),
                            op=mybir.AluOpType.add)
                    if extras and h < len(extras):
                        extras[h]()

            def c_conv(s):
                # conv2 + bn2 -> z (fp32), with per-channel sums for CBAM avg.
                # Per-chunk sum accum + max reduce so the pools pipeline with
                # the matmuls instead of serializing after the full tile.
                zt = [zp.tile([128, N], F32R, tag=f"z{ob}", name=f"z{ob}")
                      for ob in range(2)]
                cols = [zp.tile([128, 4], F32, tag=f"cols{ob}", name=f"cols{ob}")
                        for ob in range(2)]
                for ob in range(2):
                    for nch in range(2):
                        pa = ps_a.tile([128, 512], F32, tag="pa", name="pa")
                        for kb in range(2):
                            nc.tensor.matmul(
                                pa, w2t[:, kb, ob * 128:(ob + 1) * 128],
                                ytmp[s][kb][:, nch * 512:(nch + 1) * 512],
                                start=(kb == 0), stop=(kb == 1))
                        if s == 1 and ob == 0:
                            nc.scalar.activation(
                                out=zt[ob][:, nch * 512:(nch + 1) * 512],
                                in_=pa, bias=sh2[:, ob:ob + 1],
                                func=mybir.ActivationFunctionType.Identity,
                                accum_out=cols[ob][:, nch:nch + 1])
                        else:
                            nc.vector.tensor_scalar(
                                zt[ob][:, nch * 512:(nch + 1) * 512], pa,
                                sh2[:, ob:ob + 1], 0.0,
                                mybir.AluOpType.add, mybir.AluOpType.add,
                                accum_out=cols[ob][:, nch:nch + 1])
                        nc.vector.tensor_reduce(
                            out=cols[ob][:, 2 + nch:3 + nch],
                            in_=zt[ob][:, nch * 512:(nch + 1) * 512],
                            op=mybir.AluOpType.max,
                            axis=mybir.AxisListType.X)
                    # combine chunk sums into col 0, chunk maxes into col 1
                    nc.vector.tensor_tensor(
                        out=cols[ob][:, 0:1], in0=cols[ob][:, 0:1],
                        in1=cols[ob][:, 1:2], op=mybir.AluOpType.add)
                    nc.vector.tensor_tensor(
                        out=cols[ob][:, 1:2], in0=cols[ob][:, 2:3],
                        in1=cols[ob][:, 3:4], op=mybir.AluOpType.max)
                return zt, cols

            def c_chan(s, zt, cols):
                # channel attention: h = relu(W1a@sum) + relu(W1m@max),
                # ca = sig(W2@h)
                ph = ps_a.tile([128, 512], F32, tag="pa", name="pa")
                for j in range(2):
                    for kb in range(2):
                        nc.tensor.matmul(ph[0:R, j:j + 1],
                                         cw1[:, kb, j * R:(j + 1) * R],
                                         cols[kb][:, j:j + 1],
                                         start=(kb == 0), stop=(kb == 1))
                hsb = sp.tile([R, 3], F32, tag="hsb", name="hsb")
                nc.vector.tensor_scalar(hsb[:, 0:2], ph[0:R, 0:2], 0.0, None,
                                        mybir.AluOpType.max)
                nc.vector.tensor_tensor(out=hsb[:, 2:3], in0=hsb[:, 0:1],
                                        in1=hsb[:, 1:2],
                                        op=mybir.AluOpType.add)
                ca = [sp.tile([128, 1], F32R, tag=f"ca{ob}", name=f"ca{ob}")
                      for ob in range(2)]
                pc = ps_a.tile([128, 512], F32, tag="pa", name="pa")
                for ob in range(2):
                    nc.tensor.matmul(pc[:, ob:ob + 1],
                                     cw2[:, ob * 128:(ob + 1) * 128],
                                     hsb[:, 2:3], start=True, stop=True)
                for ob in range(2):
                    nc.scalar.activation(
                        out=ca[ob], in_=pc[:, ob:ob + 1],
                        func=mybir.ActivationFunctionType.Sigmoid)
                # apply channel attention -> z_ca (f32r for the ones-matmul).
                # ob=0 on DVE, ob=1 on ACT (Copy with per-partition scale AP)
                # so the two run in parallel.
                zca = [zp.tile([128, N], F32R, tag=f"zca{ob}", name=f"zca{ob}")
                       for ob in range(2)]
                nc.vector.tensor_scalar_mul(zca[0], zt[0],
                                            ca[0].bitcast(F32))
                nc.scalar.activation(
                    out=zca[1], in_=zt[1],
                    func=mybir.ActivationFunctionType.Copy,
                    scale=ca[1].bitcast(F32))
                return zca, ca

            def c_spat(s, zt, zca, ca):
                # spatial sum (avg path): ca^T @ z, so it does not wait on
                # the zca tiles
                avg_row = sp.tile([1, N], F32, tag="avg_row", name="avg_row", bufs=1)
                for nch in range(2):
                    psr = ps_a.tile([128, 512], F32, tag="pa", name="pa")
                    for kb in range(2):
                        nc.tensor.matmul(
                            psr[0:1, :],
                            ca[kb], zt[kb][:, nch * 512:(nch + 1) * 512],
                            start=(kb == 0), stop=(kb == 1))
                    nc.scalar.copy(
                        avg_row[:, nch * 512:(nch + 1) * 512], psr[0:1, :])
                # spatial max via TT max + gpsimd cross-partition reduce
                m1 = zp.tile([128, N], F32, tag="m1", name="m1", bufs=1)
                nc.vector.tensor_tensor(out=m1, in0=zca[0].bitcast(F32),
                                        in1=zca[1].bitcast(F32),
                                        op=mybir.AluOpType.max)
                rep = zp.tile([128, N], F32, tag="rep", name="rep", bufs=1)
                nc.gpsimd.partition_all_reduce(rep, m1, channels=128,
                                               reduce_op=bass_isa.ReduceOp.max)
                # reshape rows [1, 1024] -> [32(y), 32(x)]: direct sbuf->sbuf
                avgT = sp.tile([32, 38], F32, tag="avgT", name="avgT")
                nc.vector.memset(avgT, 0.0)
                nc.sync.dma_start(out=avgT[:, 3:35], in_=avg_row)
                maxT = sp.tile([32, 38], F32, tag="maxT", name="maxT")
                nc.vector.memset(maxT, 0.0)
                nc.scalar.dma_start(out=maxT[:, 3:35], in_=rep[0:1, :])
                # 7x7 conv as 14 banded matmuls over y, x-shifts on free dim
                psa = ps_a.tile([128, 512], F32, tag="pa", name="pa")
                first = True
                for c2, inp in ((0, avgT), (1, maxT)):
                    for kx in range(7):
                        nc.tensor.matmul(
                            psa[0:32, 0:32],
                            wbd[:, c2 * 7 + kx, :],
                            inp[:, kx:kx + 32],
                            start=first, stop=(c2 == 1 and kx == 6))
                        first = False
                sasb = sp.tile([32, 32], F32, tag="sasb", name="sasb")
                nc.scalar.activation(
                    out=sasb, in_=psa[0:32, 0:32],
                    func=mybir.ActivationFunctionType.Sigmoid)
                sa_row = sp.tile([1, N], F32, tag="sa_row", name="sa_row",
                                 bufs=2)
                nc.sync.dma_start(out=sa_row, in_=sasb)
                sarep = zp.tile([128, N], F32, tag="sarep", name="sarep", bufs=1)
                nc.gpsimd.partition_broadcast(sarep, sa_row, channels=128)
                return sarep

            def c_fin(s, zca, sarep):
                # final: out = relu(z_ca * sa + x). In the s=1 tail, split
                # the four 512-chunks across DVE and Pool so they overlap.
                for cb in range(2):
                    t = op_.tile([128, N], F32, tag="fin", name="fin")
                    o = op_.tile([128, N], F32, tag="fino", name="fino")
                    for ci, (lo, hi) in enumerate(((0, 512), (512, N))):
                        pool_chunk = (s == 0 and ci == 1) or \
                            (s == 1 and cb == 1 and ci == 1)
                        ve = nc.gpsimd if pool_chunk else nc.vector
                        ve.tensor_tensor(
                            out=t[:, lo:hi], in0=zca[cb].bitcast(F32)[:, lo:hi],
                            in1=sarep[:, lo:hi], op=mybir.AluOpType.mult)
                        ve.tensor_tensor(
                            out=t[:, lo:hi], in0=t[:, lo:hi],
                            in1=xt[s][cb].bitcast(F32)[:, lo:hi],
                            op=mybir.AluOpType.add)
                        if s == 0:
                            nc.vector.tensor_scalar(
                                o[:, lo:hi], t[:, lo:hi], 0.0, None,
                                mybir.AluOpType.max)
                        else:
                            nc.scalar.activation(
                                out=o[:, lo:hi], in_=t[:, lo:hi],
                                func=mybir.ActivationFunctionType.Relu)
                        oq = nc.scalar if s == 0 else nc.sync
                        oq.dma_start(out=out[s, cb][:, lo:hi],
                                     in_=o[:, lo:hi])

            def phase_c(s):
                zt, cols = c_conv(s)
                zca, ca = c_chan(s, zt, cols)
                sarep = c_spat(s, zt, zca, ca)
                c_fin(s, zca, sarep)

            phase_a(0)
            phase_b(0, extras=(lambda: a_conv(1), lambda: a_q(1),
                               lambda: a_vt(1), lambda: a_k(1)))
            c0_state = {}

            def x0():
                c0_state["zt"], c0_state["cols"] = c_conv(0)

            def x1():
                c0_state["zca"], c0_state["ca"] = c_chan(
                    0, c0_state["zt"], c0_state["cols"])

            def x2():
                c0_state["sarep"] = c_spat(
                    0, c0_state["zt"], c0_state["zca"], c0_state["ca"])

            def x3():
                c_fin(0, c0_state["zca"], c0_state["sarep"])

            phase_b(1, extras=(x0, x1, x2, x3))
            phase_c(1)

    nc.compile()
    return nc


_NC_CACHE = None


def get_module():
    global _NC_CACHE
    if _NC_CACHE is None:
        _NC_CACHE = build_module()
    return _NC_CACHE


def prep_inputs(x, w1, bn1_g, bn1_b, bn1_m, bn1_v, wq, bq, wk, bk, wv, bv,
                gamma, w2, bn2_g, bn2_b, bn2_m, bn2_v, ca_w1, ca_w2, sa_w):
    """Host-side preprocessing -> per-core in_maps."""
    f64 = np.float64
    s1 = (bn1_g.astype(f64) / np.sqrt(bn1_v.astype(f64) + EPS))
    w1f = (s1[:, None] * w1.astype(f64)).astype(np.float32)
    sh1 = (bn1_b.astype(f64) - bn1_m.astype(f64) * s1).astype(np.float32)
    s2 = (bn2_g.astype(f64) / np.sqrt(bn2_v.astype(f64) + EPS))
    w2f = (s2[:, None] * w2.astype(f64)).astype(np.float32)
    sh2 = (bn2_b.astype(f64) - bn2_m.astype(f64) * s2).astype(np.float32)
    g = float(gamma[0])
    wvg = (wv.astype(f64) * g).astype(np.float32)
    bvg = (bv.astype(f64) * g).astype(np.float32)

    def lhsT(w):  # [O, C] -> [2, 128, O] kb-blocked transpose, tf32
        return tf32_round(np.ascontiguousarray(
            w.T.reshape(2, 128, C)))

    # q/k out-channel permutation for DoubleRow scores:
    # free position sub*128 + h*32 + dlo <- channel h*64 + sub*32 + dlo
    perm = np.empty(C, np.int64)
    for h_ in range(NH):
        for sub in range(2):
            for dlo in range(32):
                perm[sub * 128 + h_ * 32 + dlo] = h_ * 64 + sub * 32 + dlo
    # wall[p, j, kb, c]: stationary weights, kb-blocked transpose
    wall_np = np.stack([lhsT(w1f), lhsT(wq[perm]), lhsT(wk[perm]),
                        lhsT(wvg), lhsT(w2f)], axis=0)  # [5, 2, 128, C]
    wall_np = np.ascontiguousarray(wall_np.transpose(2, 0, 1, 3))
    sm = np.zeros((128, 10), np.float32)
    sm[:, 0:2] = sh1.reshape(2, 128).T
    sm[:, 2:4] = bq[perm].reshape(2, 128).T
    sm[:, 4:6] = bk[perm].reshape(2, 128).T
    sm[:, 6:8] = sh2.reshape(2, 128).T
    sm[:, 8] = 1.0
    base = {
        "wall": wall_np,
        "smalls": sm,
        "bv_r": np.ascontiguousarray(bvg.reshape(1, C)),
    }
    # channel attention weights: caw1T [2, 128, 64]
    c1T = ca_w1.T.astype(np.float32)             # [C, R]
    caw1T = np.concatenate([c1T / float(N), c1T], axis=1)  # [C, 2R]
    base["caw1T"] = np.ascontiguousarray(caw1T.reshape(2, 128, 2 * R))
    caw2T = np.ascontiguousarray(ca_w2.T.astype(np.float32))  # [R, C]
    # spatial conv bands: wband[yi, c2*7+kx, yo] = w[c2, yi-yo+3, kx]
    wb = np.zeros((32, 14, 32), np.float32)
    for c2 in range(2):
        for kx in range(7):
            for yo in range(32):
                for ky in range(7):
                    yi = yo + ky - 3
                    if 0 <= yi < 32:
                        v = sa_w[0, c2, ky, kx]
                        if c2 == 0:
                            v = v / float(C)
                        wb[yi, c2 * 7 + kx, yo] = v
    base["cwb"] = np.concatenate([caw2T, wb.reshape(32, 14 * 32)], axis=1)
    import ml_dtypes
    base["ident"] = np.eye(128, dtype=ml_dtypes.bfloat16)

    xrf = tf32_round(x.reshape(B, C, N))
    in_maps = []
    for core in range(NCORES):
        m = dict(base)
        m["xr"] = np.ascontiguousarray(
            xrf[core * SPC:(core + 1) * SPC].reshape(SPC, 2, 128, N))
        in_maps.append(m)
    return in_maps


def kernel(**inputs):
    nc = get_module()
    in_maps = prep_inputs(**inputs)
    res = run_bass_kernel_spmd(nc, in_maps, core_ids=list(range(NCORES)))
    outs = []
    for core in range(NCORES):
        o = res.results[core]["out"]  # [SPC, 2, 128, N]
        outs.append(o.reshape(SPC, C, H, W))
    return np.concatenate(outs, axis=0)


if __name__ == "__main__":
    nc = get_module()
    print("compiled ok")

